# revision 1
# baseline (speedup 1.0000x reference)
"""Trainium2 Bass kernel for nn_FChCombxValEncoder (HDC n-gram encoder).

Computation: idx = quantize(x) -> signal = signals_weight[idx] -> bind with
feat_weight -> 4-gram product with per-step D-rolls -> bundle sum over n-grams
-> hard sign.

Distribution: the feature axis (n_feat = 4096) is sharded across 8 cores
(512 n-gram starts each, +3 halo rows); the bundle sum is AllReduced.

Device pipeline per core (all values are +/-1 so bf16 is exact):
  - 5 t-blocks; per block: one indirect-DMA gather of the level rows (full
    padded width 10003 = D + 3 wrap cols baked in on host) + one feat DMA.
  - per block x 10 D-chunks (W=1000): DVE S = sig*feat; PE shift-matmuls
    provide the t+1 / t+2 partition shifts (exact for +/-1 data); ACT copies
    PSUM->SBUF bf16 absorbing the odd D-shift so DVE TT ops stay in 2x mode;
    DVE U = S . S1, Q = U . U2; ones-matmul reduces over t into PSUM.
  - block partials staged in quadrant rows {0,32,64,96} of an SBUF f32
    accumulator (the 12-row runt block is DVE-added into row 0), summed by a
    final ones-matmul per chunk, written to DRAM, AllReduced over the 8
    cores, then sign + roll-by-3 on the way out.

The index quantization is reproduced bit-exactly via a host-precomputed
fp32 threshold table: idx = #{k : x >= b_k} where b_k is the smallest fp32
whose round(div(...)) pipeline lands at level k (fp32 ops are monotone, so
the step function is exactly representable by thresholds).
"""
import sys

sys.path.insert(0, "/opt/trn_rl_repo")

import numpy as np
import ml_dtypes

import concourse.bass as bass
import concourse.bacc as bacc
import concourse.tile as tile
import concourse.mybir as mybir
from concourse.bass_utils import run_bass_kernel_spmd

# ---- problem constants (hardcoded per contest rules) ----
MAX_VAL = 52000.0
MIN_VAL = -53000.0
NUM_LEVELS = 1000
NGRAM = 4
D = 10000
TIMESTAMPS = 128
CHANNELS = 32
NFEAT = TIMESTAMPS * CHANNELS          # 4096
NCORE = 8
L = NFEAT - (NGRAM - 1)                # 4093 n-grams total

DP = D + 3                             # padded row width (wrap cols baked in)
W = 1000                               # D-chunk width
NCHUNK = D // W                        # 10
PER_CORE = 512                         # n-gram starts per core (core 7: 509 via zero-pad)
ROWS = PER_CORE + 3                    # 515 rows needed per core
Q0 = [0, 125, 250, 375, 500]           # block q-row starts
NLOAD = [128, 128, 128, 128, 15]       # rows loaded per block
NQ = [125, 125, 125, 125, 12]          # valid n-gram rows per block
ROLL = NGRAM - 1                       # final roll amount

F32 = mybir.dt.float32
BF16 = mybir.dt.bfloat16
I32 = mybir.dt.int32

_BF = ml_dtypes.bfloat16


# ---------------------------------------------------------------- host prep
def _f2o(u):
    """fp32 bits -> order-preserving int64."""
    b = u.view(np.uint32).astype(np.int64)
    return np.where(b < 0x80000000, b + 0x80000000, 0xFFFFFFFF - b)


def _o2f(o):
    b = np.where(o >= 0x80000000, o - 0x80000000, 0xFFFFFFFF - o).astype(np.uint64)
    return b.astype(np.uint32).view(np.float32)


def _g(v):
    """The reference's value_to_index pipeline, fp32 step by step."""
    v = v.astype(np.float32)
    t = (v - np.float32(MIN_VAL)).astype(np.float32)
    t = (t / np.float32(MAX_VAL - MIN_VAL)).astype(np.float32)
    t = (t * np.float32(NUM_LEVELS - 1)).astype(np.float32)
    r = np.round(t)                     # round-half-even, fp32
    return np.clip(r, 0.0, float(NUM_LEVELS - 1))


def _thresholds():
    """b_k = smallest fp32 v with _g(v) >= k, for k = 1..999 (monotone bisection
    on the fp32 ordered-int grid)."""
    ks = np.arange(1, NUM_LEVELS, dtype=np.float32)
    lo = _f2o(np.full(ks.shape, np.float32(MIN_VAL) - np.float32(2.0)))
    hi = _f2o(np.full(ks.shape, np.float32(MAX_VAL) + np.float32(2.0)))
    # invariants: g(lo) < k <= g(hi)
    for _ in range(64):
        mid = (lo + hi) // 2
        vm = _o2f(mid)
        ge = _g(vm) >= ks
        hi = np.where(ge, mid, hi)
        lo = np.where(ge, lo, mid)
        if np.all(hi - lo <= 1):
            break
    return _o2f(hi)                     # (999,) fp32


def _shift_mat(n, s):
    m = np.zeros((n, n), dtype=_BF)
    for i in range(n - s):
        m[i + s, i] = 1.0
    return m


_CACHE = {}


def _host_constants():
    if "thr" not in _CACHE:
        thr = _thresholds()
        _CACHE["thr"] = np.tile(thr[None, :], (128, 1)).astype(np.float32)
        _CACHE["sh1"] = _shift_mat(128, 1)
        _CACHE["sh2"] = _shift_mat(128, 2)
        _CACHE["ones_red"] = np.ones((128, 1), dtype=_BF)
        ones4 = np.zeros((128, 1), dtype=np.float32)
        ones4[[0, 32, 64, 96], 0] = 1.0
        _CACHE["ones4"] = ones4
    return _CACHE


# ---------------------------------------------------------------- program
def _build_program():
    nc = bacc.Bacc("TRN2", target_bir_lowering=False, debug=False,
                   num_devices=NCORE)

    x_d = nc.dram_tensor("x_blocks", (128, 5), F32, kind="ExternalInput")
    thr_d = nc.dram_tensor("thr", (128, NUM_LEVELS - 1), F32, kind="ExternalInput")
    table_d = nc.dram_tensor("table", (NUM_LEVELS, DP), BF16, kind="ExternalInput")
    feat_d = nc.dram_tensor("feat", (ROWS, DP), BF16, kind="ExternalInput")
    sh1_d = nc.dram_tensor("sh1", (128, 128), BF16, kind="ExternalInput")
    sh2_d = nc.dram_tensor("sh2", (128, 128), BF16, kind="ExternalInput")
    onr_d = nc.dram_tensor("ones_red", (128, 1), BF16, kind="ExternalInput")
    on4_d = nc.dram_tensor("ones4", (128, 1), F32, kind="ExternalInput")
    out_d = nc.dram_tensor("out", (1, D), F32, kind="ExternalOutput")

    cc_in = nc.dram_tensor("cc_in", (1, D), F32)
    cc_out = nc.dram_tensor("cc_out", (1, D), F32, addr_space="Shared")

    NTH = NUM_LEVELS - 1

    with tile.TileContext(nc) as tc:
        with tc.tile_pool(name="const", bufs=1) as cpool, \
             tc.tile_pool(name="loads", bufs=2) as lpool, \
             tc.tile_pool(name="work", bufs=2) as wpool, \
             tc.tile_pool(name="accum", bufs=1) as apool, \
             tc.tile_pool(name="psh", bufs=2, space="PSUM") as psh, \
             tc.tile_pool(name="pacc", bufs=2, space="PSUM") as pacc:

            sh1 = cpool.tile([128, 128], BF16)
            nc.sync.dma_start(out=sh1[:, :], in_=sh1_d[:, :])
            sh2 = cpool.tile([128, 128], BF16)
            nc.sync.dma_start(out=sh2[:, :], in_=sh2_d[:, :])
            onr = cpool.tile([128, 1], BF16)
            nc.sync.dma_start(out=onr[:, :], in_=onr_d[:, :])
            on4 = cpool.tile([128, 1], F32)
            nc.sync.dma_start(out=on4[:, :], in_=on4_d[:, :])
            thr = cpool.tile([128, NTH], F32)
            nc.sync.dma_start(out=thr[:, :], in_=thr_d[:, :])
            xall = cpool.tile([128, 5], F32)
            nc.sync.dma_start(out=xall[:, :], in_=x_d[:, :])

            # P2: block-partial accumulator, quadrant rows 0/32/64/96 (+runt
            # added into row 0). memset kills NaN risk from uninit SBUF.
            P2 = apool.tile([128, D], F32)
            nc.vector.memset(P2[:, :], 0.0)

            # ---- per-block indices ----
            idx_tiles = []
            for b in range(5):
                ge = wpool.tile([128, NTH], F32, tag="ge")
                nc.vector.tensor_tensor(
                    out=ge[:, :], in0=xall[:, b:b + 1].to_broadcast([128, NTH]),
                    in1=thr[:, :], op=mybir.AluOpType.is_ge)
                idxf = wpool.tile([128, 1], F32, tag="idxf")
                nc.vector.tensor_reduce(out=idxf[:, :], in_=ge[:, :],
                                        axis=mybir.AxisListType.X,
                                        op=mybir.AluOpType.add)
                it = cpool.tile([128, 1], I32, tag=f"idx{b}")
                nc.vector.tensor_copy(out=it[:, :], in_=idxf[:, :])
                idx_tiles.append(it)

            # ---- main loop: block outer, chunk inner ----
            for b in [0, 4, 1, 2, 3]:
                nl, nq, q0 = NLOAD[b], NQ[b], Q0[b]
                nu = nl - 1                       # valid U rows

                sig = lpool.tile([128, DP], BF16, tag="sig")
                nc.gpsimd.indirect_dma_start(
                    out=sig[0:nl, :], out_offset=None,
                    in_=table_d[:, :],
                    in_offset=bass.IndirectOffsetOnAxis(
                        ap=idx_tiles[b][0:nl, 0:1], axis=0),
                )
                fe = lpool.tile([128, DP], BF16, tag="fe")
                nc.sync.dma_start(out=fe[0:nl, :], in_=feat_d[q0:q0 + nl, :])

                for c in range(NCHUNK):
                    e0 = c * W
                    wp = W + 3                    # S chunk width

                    s_t = wpool.tile([128, wp], BF16, tag="s")
                    nc.vector.tensor_tensor(
                        out=s_t[0:nl, :], in0=sig[0:nl, e0:e0 + wp],
                        in1=fe[0:nl, e0:e0 + wp], op=mybir.AluOpType.mult)

                    # S1[m, :] = S[m+1, :]
                    s1p = psh.tile([128, wp], F32, tag="shift")
                    for a0, a1 in ((0, 512), (512, wp)):
                        nc.tensor.matmul(out=s1p[0:nu, a0:a1],
                                         lhsT=sh1[0:nl, 0:nu],
                                         rhs=s_t[0:nl, a0:a1],
                                         start=True, stop=True)
                    s1s = wpool.tile([128, W + 2], BF16, tag="s1s")
                    nc.scalar.copy(out=s1s[0:nu, :], in_=s1p[0:nu, 1:wp])

                    # U[t, e] = S[t, e] * S[t+1, e+1]
                    u_t = wpool.tile([128, W + 2], BF16, tag="u")
                    nc.vector.tensor_tensor(
                        out=u_t[0:nu, :], in0=s_t[0:nu, 0:W + 2],
                        in1=s1s[0:nu, :], op=mybir.AluOpType.mult)

                    # U2[m, :] = U[m+2, :]
                    u2p = psh.tile([128, wp], F32, tag="shift")
                    for a0, a1 in ((0, 512), (512, W + 2)):
                        nc.tensor.matmul(out=u2p[0:nq, a0:a1],
                                         lhsT=sh2[0:nu, 0:nq],
                                         rhs=u_t[0:nu, a0:a1],
                                         start=True, stop=True)
                    u2s = wpool.tile([128, W], BF16, tag="u2s")
                    nc.scalar.copy(out=u2s[0:nq, :], in_=u2p[0:nq, 2:W + 2])

                    # Q[t, e] = U[t, e] * U[t+2, e+2]
                    q_t = wpool.tile([128, W], BF16, tag="q")
                    nc.vector.tensor_tensor(
                        out=q_t[0:nq, :], in0=u_t[0:nq, 0:W],
                        in1=u2s[0:nq, :], op=mybir.AluOpType.mult)

                    # bundle partial: sum over t rows
                    accp = pacc.tile([1, W], F32, tag="acc")
                    for a0, a1 in ((0, 512), (512, W)):
                        nc.tensor.matmul(out=accp[0:1, a0:a1],
                                         lhsT=onr[0:nq, 0:1],
                                         rhs=q_t[0:nq, a0:a1],
                                         start=True, stop=True)
                    if b != 4:
                        row = 32 * b
                        nc.scalar.copy(out=P2[row:row + 1, e0:e0 + W],
                                       in_=accp[0:1, :])
                    else:
                        # runt block: accumulate into row 0 (after block 0)
                        nc.vector.tensor_tensor(
                            out=P2[0:1, e0:e0 + W], in0=P2[0:1, e0:e0 + W],
                            in1=accp[0:1, :], op=mybir.AluOpType.add)

            # ---- per-chunk quadrant sum -> cc_in ----
            for c in range(NCHUNK):
                e0 = c * W
                acc2 = pacc.tile([1, W], F32, tag="acc")
                for a0, a1 in ((0, 512), (512, W)):
                    nc.tensor.matmul(out=acc2[0:1, a0:a1],
                                     lhsT=on4[:, 0:1],
                                     rhs=P2[:, e0 + a0:e0 + a1],
                                     start=True, stop=True)
                stg = wpool.tile([1, W], F32, tag="stg")
                nc.scalar.copy(out=stg[:, :], in_=acc2[0:1, :])
                nc.sync.dma_start(out=cc_in[0:1, e0:e0 + W], in_=stg[:, :])

            # ---- AllReduce over the 8 cores ----
            nc.gpsimd.collective_compute(
                "AllReduce", mybir.AluOpType.add,
                ins=[cc_in[:, :]], outs=[cc_out[:, :]],
                replica_groups=[list(range(NCORE))],
            )

            # ---- sign + roll-by-3 output ----
            PR, PW = 125, 80
            r = wpool.tile([PR, PW], F32, tag="fin")
            nc.sync.dma_start(
                out=r[:, :],
                in_=cc_out[:, :].rearrange("o (p w) -> (o p) w", p=PR))
            t1 = wpool.tile([PR, PW], F32, tag="fin2")
            nc.vector.tensor_scalar(out=t1[:, :], in0=r[:, :], scalar1=0.0,
                                    scalar2=2.0, op0=mybir.AluOpType.is_gt,
                                    op1=mybir.AluOpType.mult)
            sg = wpool.tile([PR, PW], F32, tag="fin3")
            nc.vector.tensor_scalar(out=sg[:, :], in0=t1[:, :], scalar1=-1.0,
                                    scalar2=None, op0=mybir.AluOpType.add)
            nfull = (D - ROLL) // PW               # 124
            rem = D - ROLL - nfull * PW            # 77
            nc.sync.dma_start(out=out_d[0:1, ROLL:ROLL + nfull * PW],
                              in_=sg[0:nfull, :])
            nc.sync.dma_start(out=out_d[0:1, ROLL + nfull * PW:D],
                              in_=sg[nfull:nfull + 1, 0:rem])
            nc.sync.dma_start(out=out_d[0:1, 0:ROLL],
                              in_=sg[nfull:nfull + 1, rem:PW])

    nc.compile()
    return nc


TRACE = False
LAST_RESULT = None


def kernel(x, signals_weight, feat_weight):
    global LAST_RESULT
    consts = _host_constants()

    if "nc" not in _CACHE:
        _CACHE["nc"] = _build_program()
    nc = _CACHE["nc"]

    xf = np.asarray(x, dtype=np.float32).reshape(-1)
    sw = np.asarray(signals_weight, dtype=np.float32)
    fw = np.asarray(feat_weight, dtype=np.float32)

    table = np.empty((NUM_LEVELS, DP), dtype=_BF)
    table[:, :D] = sw.astype(_BF)
    table[:, D:] = table[:, :3]

    in_maps = []
    for m in range(NCORE):
        base = PER_CORE * m
        nreal = min(ROWS, NFEAT - base)
        fr = np.zeros((ROWS, DP), dtype=_BF)
        fr[:nreal, :D] = fw[base:base + nreal].astype(_BF)
        fr[:nreal, D:] = fr[:nreal, :3]

        xr = np.full(640, xf[-1], dtype=np.float32)
        xr[:nreal] = xf[base:base + nreal]
        xb = np.lib.stride_tricks.as_strided(
            xr, shape=(128, 5), strides=(4, 500)).copy()

        in_maps.append({
            "x_blocks": xb,
            "thr": consts["thr"],
            "table": table,
            "feat": fr,
            "sh1": consts["sh1"],
            "sh2": consts["sh2"],
            "ones_red": consts["ones_red"],
            "ones4": consts["ones4"],
        })

    res = run_bass_kernel_spmd(nc, in_maps, list(range(NCORE)), trace=TRACE)
    LAST_RESULT = res
    return np.asarray(res.results[0]["out"], dtype=np.float32)
